# revision 1
# baseline (speedup 1.0000x reference)
"""MoNCE loss (OT-regularized InfoNCE) Trainium2 kernel.

Data-parallel over the 8 independent OT problems, 1 per NeuronCore.
Per core (N=2048 patches, D=256), with T = NCE temperature:

  Merged K/Sinkhorn loop (1 iteration suffices: truncation ~1e-8 vs 50):
    per row-chunk t: K_t = exp(-qn_t.kn^T)  [bf16 matmul + ACT exp;
                     ACT accum_out -> rowsum r_t for free]
                     u_t = 1/(r_t/N + 1e-8)             [tiny per-chunk ops]
                     z += u_t^T K_t                     [PE matvec, K_t dies]
    v = 1/(z + N*1e-8)
  Fused CE via ONE augmented matmul (c = 2*D+1 contraction rows):
    S''_ij = q_i.k_j - T*(kn_i.qn_j) + T*ln(u_j)
           = [qTr; -T*knT; T*ones]^T . [kTr; qnT; ln u]
    M_i  = rowmax(S'')                     [DVE reduce from PSUM]
    A_i  = sum_j exp((S''_ij - M_i)/T)     [ACT exp accum_out]
         = sum_j K^T_ij u_j exp((S_ij - M_i)/T)
    tot  = (2047/2048) v_i (A_i - u_i Ktii_i Epos_i) + Epos_i
    loss = (M_i - S_ii)/T + ln(tot)
  (the reference's +1e-8 inside f contributes < 1e-4 absolute - dropped)
"""

import os
from contextlib import ExitStack

import numpy as np

import concourse.bass as bass
import concourse.tile as tile
from concourse import bacc, mybir
from concourse.bass_utils import run_bass_kernel_spmd

F32 = mybir.dt.float32
F32R = mybir.dt.float32r
BF16 = mybir.dt.bfloat16
AF = mybir.ActivationFunctionType
ALU = mybir.AluOpType
AX = mybir.AxisListType

N = 2048
D = 256
NCH = N // 128    # 16 row chunks
DCH = D // 128    # 2 contraction chunks
T = 0.07
EPS = 1e-8
SC = (N - 1) / N

_CACHED_NC = None


def _build():
    stage = int(os.environ.get("KSTAGE", "9"))
    nc = bacc.Bacc("TRN2", target_bir_lowering=False, debug=False, num_devices=8)

    qTd = nc.dram_tensor("qT", [D, N], F32, kind="ExternalInput").ap()
    kTd = nc.dram_tensor("kT", [D, N], F32, kind="ExternalInput").ap()
    lossd = nc.dram_tensor("loss", [N], F32, kind="ExternalOutput").ap()
    lnud = nc.dram_tensor("lnub", [N], BF16).ap()
    siid = nc.dram_tensor("siib", [N], F32).ap()
    vbd = nc.dram_tensor("vb", [N], BF16).ap()
    riqd = nc.dram_tensor("riqb", [N], BF16).ap()
    rikd = nc.dram_tensor("rikb", [N], BF16).ap()

    col_view = lambda d: d.rearrange("(t p) -> p t", p=128)
    row_view = lambda d: d.rearrange("(a n) -> a n", a=1)

    with tile.TileContext(nc) as tc, ExitStack() as ctx:
        sg = ctx.enter_context(tc.tile_pool(name="sg", bufs=1))
        io = ctx.enter_context(tc.tile_pool(name="io", bufs=2))
        scr = ctx.enter_context(tc.tile_pool(name="scr", bufs=3))
        sqp = ctx.enter_context(tc.tile_pool(name="sqp", bufs=4))
        prp = ctx.enter_context(tc.tile_pool(name="prp", bufs=2))
        kcp = ctx.enter_context(tc.tile_pool(name="kcp", bufs=6))
        ps = ctx.enter_context(tc.tile_pool(name="ps", bufs=4, space="PSUM"))

        # ---------------- constants ----------------
        ones_f = sg.tile([1, 128], F32)
        nc.vector.memset(ones_f[:], 1.0)
        ones_row = sg.tile([1, 128], BF16)
        nc.vector.tensor_copy(ones_row[:], ones_f[:])
        tee_row = sg.tile([1, 128], F32)
        nc.vector.memset(tee_row[:], T)
        tee_row16 = sg.tile([1, 128], BF16)
        nc.vector.tensor_copy(tee_row16[:], tee_row[:])
        onec_f = sg.tile([128, 1], F32)
        nc.vector.memset(onec_f[:], 1.0)
        onec_16 = sg.tile([128, 1], BF16)
        nc.vector.tensor_copy(onec_16[:], onec_f[:])
        onec_r = sg.tile([128, 1], F32R)
        nc.vector.tensor_copy(onec_r[:], onec_f[:])

        # ---------------- transposed loads + row stats ----------------
        qTr = sg.tile([128, DCH, N], F32R)   # fp32r rounded
        kTr = sg.tile([128, DCH, N], F32R)
        sqq = []
        sqk = []
        prod = []
        dma_engs = [nc.sync, nc.scalar, nc.gpsimd, nc.sync]
        for c in range(DCH):
            qtch = io.tile([128, N], F32, tag="tch")
            dma_engs[2 * c].dma_start(qtch[:], qTd[c * 128:(c + 1) * 128, :])
            nc.vector.tensor_copy(qTr[:, c, :], qtch[:])
            sq = sqp.tile([128, N], BF16, tag="sq")
            nc.scalar.activation(sq[:], qtch[:], AF.Square)
            sqq.append(sq)
            ktch = io.tile([128, N], F32, tag="tch")
            dma_engs[2 * c + 1].dma_start(ktch[:], kTd[c * 128:(c + 1) * 128, :])
            nc.vector.tensor_copy(kTr[:, c, :], ktch[:])
            sk = sqp.tile([128, N], BF16, tag="sq")
            nc.scalar.activation(sk[:], ktch[:], AF.Square)
            sqk.append(sk)
            pr = prp.tile([128, N], F32R, tag="prod")
            nc.vector.tensor_mul(pr[:], qtch[:], ktch[:])
            prod.append(pr)

        # PE ones-reductions over d -> row stats [1, N]
        sqn_q = sg.tile([1, N], F32)   # sqrt(sum q^2)
        sqn_k = sg.tile([1, N], F32)
        sii_r = sg.tile([1, N], F32, tag="rowtmp")
        for ff in range(4):
            fs = slice(ff * 512, (ff + 1) * 512)
            pq = ps.tile([1, 512], F32, tag="ps")
            pk = ps.tile([1, 512], F32, tag="ps")
            pss = ps.tile([1, 512], F32, tag="ps")
            for c in range(DCH):
                nc.tensor.matmul(pq[0:1, :], onec_16[:], sqq[c][:, fs],
                                 start=(c == 0), stop=(c == DCH - 1))
                nc.tensor.matmul(pk[0:1, :], onec_16[:], sqk[c][:, fs],
                                 start=(c == 0), stop=(c == DCH - 1))
                nc.tensor.matmul(pss[0:1, :], onec_r[:], prod[c][:, fs],
                                 start=(c == 0), stop=(c == DCH - 1))
            nc.scalar.activation(sqn_q[:, fs], pq[0:1, :], AF.Sqrt)
            nc.scalar.activation(sqn_k[:, fs], pk[0:1, :], AF.Sqrt)
            nc.scalar.copy(sii_r[:, fs], pss[0:1, :])

        # rinv rows (recip in place, then bf16)
        nc.vector.reciprocal(sqn_q[:], sqn_q[:])
        nc.vector.reciprocal(sqn_k[:], sqn_k[:])
        riq_r = sg.tile([1, N], BF16)
        rik_r = sg.tile([1, N], BF16)
        nc.vector.tensor_copy(riq_r[:], sqn_q[:])
        nc.vector.tensor_copy(rik_r[:], sqn_k[:])

        # bounce row stats to column layout (epilogue-only; off critical path)
        nc.sync.dma_start(row_view(siid), sii_r[0:1, :])
        sii = sg.tile([128, NCH], F32)
        nc.sync.dma_start(sii[:], col_view(siid))
        nc.sync.dma_start(row_view(riqd), riq_r[0:1, :])
        nc.sync.dma_start(row_view(rikd), rik_r[0:1, :])
        riq_c16 = sg.tile([128, NCH], BF16)
        rik_c16 = sg.tile([128, NCH], BF16)
        nc.sync.dma_start(riq_c16[:], col_view(riqd))
        nc.sync.dma_start(rik_c16[:], col_view(rikd))

        # broadcast a bf16 row across 128 partitions via PE outer product
        def pe_broadcast(dst_bf16, src_row_bf16):
            for h in range(2):
                bc = ps.tile([128, 1024], F32, tag="ps")
                for f in range(2):
                    sl = slice(h * 1024 + f * 512, h * 1024 + (f + 1) * 512)
                    nc.tensor.matmul(bc[:, f * 512:(f + 1) * 512], ones_row[:],
                                     src_row_bf16[:, sl], start=True, stop=True)
                nc.scalar.copy(dst_bf16[:, h * 1024:(h + 1) * 1024], bc[:])

        riq_bc = sg.tile([128, N], BF16, tag="bc")
        rik_bc = sg.tile([128, N], BF16, tag="bc")
        pe_broadcast(riq_bc, riq_r)
        pe_broadcast(rik_bc, rik_r)

        # ---------------- normalized features ----------------
        qnT = sg.tile([128, DCH, N], BF16)   # row-normalized bf16
        knTT = sg.tile([128, DCH, N], BF16)  # row-normalized, scaled by -T
        for c in range(DCH):
            nc.vector.tensor_mul(qnT[:, c, :], qTr[:, c, :].bitcast(F32),
                                 riq_bc[:])
            knt = scr.tile([128, N], BF16, tag="knt")
            nc.vector.tensor_mul(knt[:], kTr[:, c, :].bitcast(F32), rik_bc[:])
            nc.vector.tensor_scalar_mul(knTT[:, c, :], knt[:], -T)

        # ---------------- merged K pass + Sinkhorn ----------------
        if stage >= 2:
            r2 = sg.tile([128, 2 * NCH], F32)     # per-half rowsums
            r_col = sg.tile([128, NCH], F32)
            u_col = sg.tile([128, NCH], F32)
            u_col16 = sg.tile([128, NCH], BF16)
            lnu_c = sg.tile([128, NCH], BF16)
            zps_a = ps.tile([1, 2, 512], F32, tag="ps")
            zps_b = ps.tile([1, 2, 512], F32, tag="ps")
            def emit_mv(t, khs):
                for f in range(4):
                    zp = zps_a if f < 2 else zps_b
                    nc.tensor.matmul(zp[0:1, f % 2, :], u_col16[:, t:t + 1],
                                     khs[f // 2][:, (f % 2) * 512:(f % 2 + 1) * 512],
                                     start=(t == 0), stop=(t == NCH - 1))

            pend = None
            for t in range(NCH):
                khs = []
                for h in range(2):
                    cps = ps.tile([128, 1024], F32, tag="ps")
                    for f in range(2):
                        fs = slice(h * 1024 + f * 512, h * 1024 + (f + 1) * 512)
                        for c in range(DCH):
                            nc.tensor.matmul(cps[:, f * 512:(f + 1) * 512],
                                             qnT[:, c, t * 128:(t + 1) * 128],
                                             knTT[:, c, fs],
                                             start=(c == 0), stop=(c == DCH - 1))
                    # cps holds -T*C ; exp(-C) = exp(cps/T)
                    kt16 = kcp.tile([128, 1024], BF16, tag="kch")
                    nc.scalar.activation(kt16[:], cps[:], AF.Exp, scale=1.0 / T,
                                         accum_out=r2[:, 2 * t + h:2 * t + h + 1])
                    khs.append(kt16)
                # u for chunk t (tiny [128,1] column ops)
                nc.vector.tensor_add(r_col[:, t:t + 1], r2[:, 2 * t:2 * t + 1],
                                     r2[:, 2 * t + 1:2 * t + 2])
                nc.scalar.activation(u_col[:, t:t + 1], r_col[:, t:t + 1],
                                     AF.Copy, bias=EPS, scale=1.0 / N)
                nc.vector.reciprocal(u_col[:, t:t + 1], u_col[:, t:t + 1])
                nc.vector.tensor_copy(u_col16[:, t:t + 1], u_col[:, t:t + 1])
                nc.scalar.activation(lnu_c[:, t:t + 1], u_col[:, t:t + 1], AF.Ln)
                # matvec for the PREVIOUS chunk (u latency hidden by this
                # chunk's matmuls); K chunk dies at its matvec
                if pend is not None:
                    emit_mv(*pend)
                pend = (t, khs)
            emit_mv(*pend)

            # v = 1/(z + N*EPS) and ln(u) row bounce
            nc.sync.dma_start(col_view(lnud), lnu_c[:])
            lnu_row = sg.tile([1, N], BF16)
            nc.sync.dma_start(lnu_row[0:1, :], row_view(lnud))
            t2 = sg.tile([1, N], BF16)
            nc.scalar.activation(t2[:, 0:1024],
                                 zps_a.rearrange("a b c -> a (b c)")[0:1, :],
                                 AF.Copy, bias=EPS * N, scale=1.0)
            nc.scalar.activation(t2[:, 1024:2048],
                                 zps_b.rearrange("a b c -> a (b c)")[0:1, :],
                                 AF.Copy, bias=EPS * N, scale=1.0)
            nc.sync.dma_start(row_view(vbd), t2[0:1, :])
            t2c = sg.tile([128, NCH], BF16)
            nc.sync.dma_start(t2c[:], col_view(vbd))
            v_col = sg.tile([128, NCH], F32)
            nc.vector.reciprocal(v_col[:], t2c[:])

        # ---------------- fused CE: augmented S'' matmul ----------------
        if stage >= 4:
            m2 = sg.tile([128, 2 * NCH], F32)
            negm2 = sg.tile([128, 2 * NCH], F32)
            a2 = sg.tile([128, 2 * NCH], F32)
            for t in range(NCH):
                for h in range(2):
                    sps = ps.tile([128, 1024], F32, tag="ps")
                    isl = slice(t * 128, (t + 1) * 128)
                    for f in range(2):
                        fs = slice(h * 1024 + f * 512, h * 1024 + (f + 1) * 512)
                        out = sps[:, f * 512:(f + 1) * 512]
                        nc.tensor.matmul(out, qTr[:, 0, isl], kTr[:, 0, fs],
                                         start=True, stop=False)
                        nc.tensor.matmul(out, qTr[:, 1, isl], kTr[:, 1, fs],
                                         start=False, stop=False)
                        nc.tensor.matmul(out, knTT[:, 0, isl], qnT[:, 0, fs],
                                         start=False, stop=False,
                                         skip_group_check=True)
                        nc.tensor.matmul(out, knTT[:, 1, isl], qnT[:, 1, fs],
                                         start=False, stop=False,
                                         skip_group_check=True)
                        nc.tensor.matmul(out, tee_row16[:], lnu_row[0:1, fs],
                                         start=False, stop=True,
                                         skip_group_check=True)
                    hh = 2 * t + h
                    nc.vector.tensor_reduce(m2[:, hh:hh + 1], sps[:], AX.X,
                                            ALU.max)
                    nc.vector.tensor_scalar_mul(negm2[:, hh:hh + 1],
                                                m2[:, hh:hh + 1], -1.0 / T)
                    esc = scr.tile([128, 1024], BF16, tag="esc")
                    nc.scalar.activation(esc[:], sps[:], AF.Exp, scale=1.0 / T,
                                         bias=negm2[:, hh:hh + 1],
                                         accum_out=a2[:, hh:hh + 1])

        # ---------------- epilogue (column layout [128, NCH]) ----------------
        if stage >= 9:
            m2v = m2.rearrange("p (t h) -> p t h", h=2)
            a2v = a2.rearrange("p (t h) -> p t h", h=2)
            mcol = sg.tile([128, NCH], F32)
            nc.vector.tensor_max(mcol[:], m2v[:, :, 0], m2v[:, :, 1])
            acol = sg.tile([128, NCH], F32)
            wh = sg.tile([128, NCH], F32)
            for h in range(2):
                dm = sg.tile([128, NCH], F32, tag="dm")
                nc.vector.tensor_sub(dm[:], m2v[:, :, h], mcol[:])
                eh = sg.tile([128, NCH], F32, tag="eh")
                nc.scalar.activation(eh[:], dm[:], AF.Exp, scale=1.0 / T)
                if h == 0:
                    nc.vector.tensor_mul(acol[:], a2v[:, :, 0], eh[:])
                else:
                    nc.vector.tensor_mul(wh[:], a2v[:, :, 1], eh[:])
            nc.vector.tensor_add(acol[:], acol[:], wh[:])

            cii = sg.tile([128, NCH], F32)
            nc.vector.tensor_mul(cii[:], sii[:], riq_c16[:])
            nc.vector.tensor_mul(cii[:], cii[:], rik_c16[:])
            ktii = sg.tile([128, NCH], F32)
            nc.scalar.activation(ktii[:], cii[:], AF.Exp, scale=-1.0)
            dcol = sg.tile([128, NCH], F32)
            nc.vector.tensor_sub(dcol[:], sii[:], mcol[:])
            epos = sg.tile([128, NCH], F32)
            nc.scalar.activation(epos[:], dcol[:], AF.Exp, scale=1.0 / T)
            diag = sg.tile([128, NCH], F32)
            nc.vector.tensor_mul(diag[:], u_col[:], ktii[:])
            nc.vector.tensor_mul(diag[:], diag[:], epos[:])
            nc.vector.tensor_sub(acol[:], acol[:], diag[:])
            nc.vector.tensor_mul(acol[:], acol[:], v_col[:])
            nc.vector.tensor_scalar_mul(acol[:], acol[:], SC)
            tot = sg.tile([128, NCH], F32)
            nc.vector.tensor_add(tot[:], acol[:], epos[:])
            lg = sg.tile([128, NCH], F32)
            nc.scalar.activation(lg[:], tot[:], AF.Ln)
            lcol = sg.tile([128, NCH], F32)
            nc.vector.tensor_scalar_mul(lcol[:], dcol[:], -1.0 / T)
            nc.vector.tensor_add(lcol[:], lcol[:], lg[:])
            nc.sync.dma_start(col_view(lossd), lcol[:])
        else:
            lcol0 = sg.tile([128, NCH], F32)
            nc.vector.tensor_copy(lcol0[:], sii[:])
            nc.sync.dma_start(col_view(lossd), lcol0[:])

    nc.compile()
    return nc


def _get_nc():
    global _CACHED_NC
    if _CACHED_NC is None:
        _CACHED_NC = _build()
    return _CACHED_NC


def kernel(feat_q, feat_k, current_batch):
    feat_q = np.ascontiguousarray(np.asarray(feat_q, dtype=np.float32))
    feat_k = np.ascontiguousarray(np.asarray(feat_k, dtype=np.float32))
    bb = int(current_batch)
    assert bb == 8 and feat_q.shape == (8 * N, D), (bb, feat_q.shape)

    nc = _get_nc()
    in_maps = []
    for b in range(8):
        q = feat_q[b * N:(b + 1) * N]
        k = feat_k[b * N:(b + 1) * N]
        in_maps.append({
            "qT": np.ascontiguousarray(q.T),
            "kT": np.ascontiguousarray(k.T),
        })
    res = run_bass_kernel_spmd(nc, in_maps, core_ids=list(range(8)))
    out = np.concatenate([res.results[b]["loss"].reshape(-1) for b in range(8)])
    return out.astype(np.float32)



# revision 9
# speedup vs baseline: 1.3119x; 1.3119x over previous
"""MoNCE loss (OT-regularized InfoNCE) Trainium2 kernel, v2.

Data-parallel over the 8 independent OT problems, 1 per NeuronCore.
Per core (N=2048 patches, D=256), with T = NCE temperature:

  Row stats in COLUMN layout [128,16] (cheap DVE reciprocal), bounced
  once through DRAM to row layout for a PE outer-product broadcast.
  Merged K/Sinkhorn loop (1 iteration suffices):
    per row-chunk t: cps = qn_t . (-T*kn)  [bf16 matmul]
                     K_t = exp(cps/T) [ACT, accum_out rowsum r_t]
                     u_t = 1/(r_t/N + 1e-8)  [DVE only]; u_t bounced
                     incrementally to a DRAM row
                     z += u_t^T K_t   [PE matvec -> one PSUM bank,
                                       f-slices at partitions 0/32/64/96]
    v = 1/(z + N*1e-8)
  Fused CE, one PSUM accumulation per 512-block:
    S''_ij = q_i.k_j - T*(kn_i.qn_j) + T*ln(u_j)
      q.k     : fp16 x fp16, 2 passes      (softmax max-cancellation
                makes per-term exp errors cancel; sim rel ~1e-4)
      kn.qn   : fp8e5m2 DoubleRow, 1 pass
      T*ln u  : bf16 rank-1 row
    M_i = rowmax (DVE), A_i = sum_j exp((S''-M_i)/T) [ACT accum]
    tot  = (2047/2048) v_i (A_i - u_i Ktii_i Epos_i) + Epos_i
    loss = (M_i - S_ii)/T + ln(tot)
All matmul weights are bf16/fp16/fp8 -> FWL stays enabled and the PE
HAM clock gate stays at 8/8 (the v1 kernel ran the whole CE phase at
half clock due to fp32r LDWEIGHTS).
"""

import os
from contextlib import ExitStack

import numpy as np

import concourse.bass as bass
import concourse.tile as tile
from concourse import bacc, mybir
from concourse.bass_utils import run_bass_kernel_spmd

F32 = mybir.dt.float32
F32R = mybir.dt.float32r
BF16 = mybir.dt.bfloat16
FP16 = mybir.dt.float16
FP8 = mybir.dt.float8e5
AF = mybir.ActivationFunctionType
ALU = mybir.AluOpType
AX = mybir.AxisListType
DR = mybir.MatmulPerfMode.DoubleRow

N = 2048
D = 256
NCH = N // 128    # 16 row chunks
DCH = D // 128    # 2 contraction chunks
T = 0.07
EPS = 1e-8
SC = (N - 1) / N

_CACHED_NC = None


def _build():
    nc = bacc.Bacc("TRN2", target_bir_lowering=False, debug=False, num_devices=8)

    qTd = nc.dram_tensor("qT", [D, N], F32, kind="ExternalInput").ap()
    kTd = nc.dram_tensor("kT", [D, N], F32, kind="ExternalInput").ap()
    lossd = nc.dram_tensor("loss", [N], F32, kind="ExternalOutput").ap()
    rvqd = nc.dram_tensor("rvqb", [N], BF16).ap()
    rvkd = nc.dram_tensor("rvkb", [N], BF16).ap()
    ubd = nc.dram_tensor("ub", [N], BF16).ap()
    siid = nc.dram_tensor("siib", [N], F32).ap()
    zbd = nc.dram_tensor("zb", [N], BF16).ap()

    col_view = lambda d: d.rearrange("(t p) -> p t", p=128)
    row_view = lambda d: d.rearrange("(a n) -> a n", a=1)

    with tile.TileContext(nc) as tc, ExitStack() as ctx:
        sg = ctx.enter_context(tc.tile_pool(name="sg", bufs=1))
        io = ctx.enter_context(tc.tile_pool(name="io", bufs=2))
        sqp = ctx.enter_context(tc.tile_pool(name="sqp", bufs=4))
        kcp = ctx.enter_context(tc.tile_pool(name="kcp", bufs=6))
        escp = ctx.enter_context(tc.tile_pool(name="escp", bufs=3))
        ps = ctx.enter_context(tc.tile_pool(name="ps", bufs=3, space="PSUM"))
        pl = ctx.enter_context(tc.tile_pool(name="pl", bufs=1, space="PSUM"))

        # ---------------- constants ----------------
        ones_f = sg.tile([1, 128], F32)
        nc.vector.memset(ones_f[:], 1.0)
        ones_row = sg.tile([1, 128], BF16)
        nc.vector.tensor_copy(ones_row[:], ones_f[:])
        negT_row = sg.tile([1, 128], BF16)
        nc.vector.memset(negT_row[:], -T)
        tee_row16 = sg.tile([1, 128], BF16)
        nc.vector.memset(tee_row16[:], T)
        onec_f = sg.tile([128, 1], F32)
        nc.vector.memset(onec_f[:], 1.0)
        onec_16 = sg.tile([128, 1], BF16)
        nc.vector.tensor_copy(onec_16[:], onec_f[:])
        onec_r = sg.tile([128, 1], F32R)
        nc.vector.tensor_copy(onec_r[:], onec_f[:])

        # ---------------- input DMA ----------------
        kTr = sg.tile([128, DCH, N], F32)    # raw kT
        qtch = sg.tile([128, DCH, N], F32)   # raw qT
        nc.sync.dma_start(kTr[:, 0, :], kTd[0:128, :])
        nc.scalar.dma_start(kTr[:, 1, :], kTd[128:256, :])
        nc.gpsimd.dma_start(qtch[:, 0, :], qTd[0:128, :])
        nc.sync.dma_start(qtch[:, 1, :], qTd[128:256, :])

        # squares (bf16) for column sumsq: c0 on ACT, c1 on DVE
        sqk = [sqp.tile([128, N], BF16, tag="sq", name=f"sqk{c}")
               for c in range(DCH)]
        sqq = [sqp.tile([128, N], BF16, tag="sq", name=f"sqq{c}")
               for c in range(DCH)]
        nc.scalar.activation(sqk[0][:], kTr[:, 0, :], AF.Square)
        nc.vector.tensor_mul(sqk[1][:], kTr[:, 1, :], kTr[:, 1, :])
        nc.scalar.activation(sqq[0][:], qtch[:, 0, :], AF.Square)
        nc.vector.tensor_mul(sqq[1][:], qtch[:, 1, :], qtch[:, 1, :])

        # ---------------- column-layout sumsq via PE (sq^T . ones) -------
        scol = ps.tile([128, 1024], F32, tag="big", name="scol")
        # [:,0:16]=k, [16:32]=q
        for b in range(NCH):
            bs = slice(b * 128, (b + 1) * 128)
            for c in range(DCH):
                nc.tensor.matmul(scol[:, b:b + 1], sqk[c][:, bs], onec_16[:],
                                 start=(c == 0), stop=(c == DCH - 1))
        for b in range(NCH):
            bs = slice(b * 128, (b + 1) * 128)
            for c in range(DCH):
                nc.tensor.matmul(scol[:, NCH + b:NCH + b + 1], sqq[c][:, bs],
                                 onec_16[:], start=(c == 0), stop=(c == DCH - 1))

        # rinv = 1/sqrt(sumsq) in column layout (k half first), bounce to rows
        rsq_col = sg.tile([128, 2 * NCH], F32)
        rinv_col = sg.tile([128, 2 * NCH], F32)
        rinv_c16 = sg.tile([128, 2 * NCH], BF16)
        nc.vector.reciprocal(rsq_col[:, 0:NCH], scol[:, 0:NCH])
        nc.scalar.activation(rinv_col[:, 0:NCH], rsq_col[:, 0:NCH], AF.Sqrt)
        nc.vector.tensor_copy(rinv_c16[:, 0:NCH], rinv_col[:, 0:NCH])
        nc.sync.dma_start(col_view(rvkd), rinv_c16[:, 0:NCH])
        rik_row = sg.tile([1, N], BF16)
        nc.sync.dma_start(rik_row[0:1, :], row_view(rvkd))
        nc.vector.reciprocal(rsq_col[:, NCH:], scol[:, NCH:2 * NCH])
        nc.scalar.activation(rinv_col[:, NCH:], rsq_col[:, NCH:], AF.Sqrt)
        nc.vector.tensor_copy(rinv_c16[:, NCH:], rinv_col[:, NCH:])
        nc.sync.dma_start(col_view(rvqd), rinv_c16[:, NCH:])
        riq_row = sg.tile([1, N], BF16)
        nc.sync.dma_start(riq_row[0:1, :], row_view(rvqd))

        # broadcast rows across partitions via PE outer product
        def pe_broadcast(dst_bf16, stat_row, src_row):
            for h in range(2):
                bc = ps.tile([128, 1024], F32, tag="big")
                for f in range(2):
                    sl = slice(h * 1024 + f * 512, h * 1024 + (f + 1) * 512)
                    nc.tensor.matmul(bc[:, f * 512:(f + 1) * 512], stat_row[:],
                                     src_row[:, sl], start=True, stop=True)
                nc.scalar.copy(dst_bf16[:, h * 1024:(h + 1) * 1024], bc[:])

        rikT_bc = sg.tile([128, N], BF16, tag="bc")   # -T / |k_j|
        riq_bc = sg.tile([128, N], BF16, tag="bc")    # 1 / |q_j|
        pe_broadcast(rikT_bc, negT_row, rik_row)
        pe_broadcast(riq_bc, ones_row, riq_row)

        # ---------------- normalized features (bf16) ----------------------
        # knTT = kT * (-T*rinvk) ; qnT = qT * rinvq  (h1 of qnT deferred)
        knTT = sg.tile([128, DCH, N], BF16)
        qnT = sg.tile([128, DCH, N], BF16)
        for c in range(DCH):
            for h in range(2):
                hs = slice(h * 1024, (h + 1) * 1024)
                nc.vector.tensor_mul(knTT[:, c, hs], kTr[:, c, hs], rikT_bc[:, hs])
        for c in range(DCH):
            nc.vector.tensor_mul(qnT[:, c, 0:1024], qtch[:, c, 0:1024],
                                 riq_bc[:, 0:1024])

        # fp16/fp8 copies + prod muls, deferred into the K loop (DVE slack)
        qT16 = sg.tile([128, DCH, N], FP16)
        kT16 = sg.tile([128, DCH, N], FP16)
        qn8 = sg.tile([128, DCH, N], FP8)
        knTT8 = sg.tile([128, DCH, N], FP8)
        prod = [io.tile([128, N], F32R, tag="prod", name=f"prod{c}")
                for c in range(DCH)]
        deferred = []
        for c in range(DCH):
            deferred.append(lambda c=c: nc.vector.tensor_mul(
                prod[c][:], qtch[:, c, :], kTr[:, c, :]))
        for c in range(DCH):
            deferred.append(lambda c=c: nc.vector.tensor_mul(
                qnT[:, c, 1024:2048], qtch[:, c, 1024:2048],
                riq_bc[:, 1024:2048]))
        for c in range(DCH):
            for h in range(2):
                hs = slice(h * 1024, (h + 1) * 1024)
                deferred.append(lambda c=c, hs=hs: nc.vector.tensor_copy(
                    knTT8[:, c, hs], knTT[:, c, hs]))
        for c in range(DCH):
            for h in range(2):
                hs = slice(h * 1024, (h + 1) * 1024)
                deferred.append(lambda c=c, hs=hs: nc.vector.tensor_copy(
                    qn8[:, c, hs], qnT[:, c, hs]))
        for c in range(DCH):
            for h in range(2):
                hs = slice(h * 1024, (h + 1) * 1024)
                deferred.append(lambda c=c, hs=hs: nc.vector.tensor_copy(
                    kT16[:, c, hs], kTr[:, c, hs]))
        for c in range(DCH):
            for h in range(2):
                hs = slice(h * 1024, (h + 1) * 1024)
                deferred.append(lambda c=c, hs=hs: nc.vector.tensor_copy(
                    qT16[:, c, hs], qtch[:, c, hs]))

        # ---------------- merged K pass + Sinkhorn ------------------------
        r2 = sg.tile([128, 2 * NCH], F32)     # per-half rowsums
        r_col = sg.tile([128, NCH], F32)
        u_col = sg.tile([128, NCH], F32)
        u_col16 = sg.tile([128, NCH], BF16)
        sii_r = sg.tile([1, N], F32, tag="rowtmp")
        sii = sg.tile([128, NCH], F32)
        # z f-slices: f=0,1,2 at partitions 0/32/64 (cols 0:512); f=3 at
        # partition 0, cols 512:1024 (quadrant 3 / partition 96 is unusable)
        zps = pl.tile([128, 1024], F32, tag="zps")
        zsl = [(0, slice(0, 512)), (32, slice(0, 512)),
               (64, slice(0, 512)), (0, slice(512, 1024))]

        def emit_mv(t, khs):
            for f in range(4):
                p, cs = zsl[f]
                nc.tensor.matmul(zps[p:p + 1, cs],
                                 u_col16[:, t:t + 1],
                                 khs[f // 2][:, (f % 2) * 512:(f % 2 + 1) * 512],
                                 start=(t == 0), stop=(t == NCH - 1))

        def emit_sii():
            # sii row reduce (f32r ones . prod), bounce row->col (epilogue)
            for ff in range(4):
                fs = slice(ff * 512, (ff + 1) * 512)
                pss = ps.tile([128, 1024], F32, tag="big")
                for c in range(DCH):
                    nc.tensor.matmul(pss[0:1, 0:512], onec_r[:], prod[c][:, fs],
                                     start=(c == 0), stop=(c == DCH - 1))
                nc.scalar.copy(sii_r[:, fs], pss[0:1, 0:512])
            nc.sync.dma_start(row_view(siid), sii_r[0:1, :])
            nc.sync.dma_start(sii[:], col_view(siid))

        pend = None
        for t in range(NCH):
            tsl = slice(t * 128, (t + 1) * 128)
            khs = []
            for h in range(2):
                cps = ps.tile([128, 1024], F32, tag="big")
                for f in range(2):
                    fs = slice(h * 1024 + f * 512, h * 1024 + (f + 1) * 512)
                    for c in range(DCH):
                        nc.tensor.matmul(cps[:, f * 512:(f + 1) * 512],
                                         qnT[:, c, tsl], knTT[:, c, fs],
                                         start=(c == 0), stop=(c == DCH - 1))
                kt16 = kcp.tile([128, 1024], BF16, tag="kch")
                nc.scalar.activation(kt16[:], cps[:], AF.Exp, scale=1.0 / T,
                                     accum_out=r2[:, 2 * t + h:2 * t + h + 1])
                khs.append(kt16)
            # u for chunk t (tiny DVE column ops, no ACT)
            nc.vector.tensor_add(r_col[:, t:t + 1], r2[:, 2 * t:2 * t + 1],
                                 r2[:, 2 * t + 1:2 * t + 2])
            nc.vector.tensor_scalar(u_col[:, t:t + 1], r_col[:, t:t + 1],
                                    1.0 / N, EPS, ALU.mult, ALU.add)
            nc.vector.reciprocal(u_col[:, t:t + 1], u_col[:, t:t + 1])
            nc.vector.tensor_copy(u_col16[:, t:t + 1], u_col[:, t:t + 1])
            # bounce u chunk to DRAM row segment (gpsimd DMA ring)
            nc.gpsimd.dma_start(
                ubd[t * 128:(t + 1) * 128].rearrange("(a p) -> p a", a=1),
                u_col16[:, t:t + 1])
            # one deferred DVE op per chunk (uses DVE slack)
            if deferred:
                deferred.pop(0)()
            # matvec for the PREVIOUS chunk (u latency hidden)
            if pend is not None:
                emit_mv(*pend)
            pend = (t, khs)
            if t == 3:
                emit_sii()
        emit_mv(*pend)
        while deferred:
            deferred.pop(0)()

        # u row readback + ln(u) row (single table switch, overlaps CE start)
        u_row = sg.tile([1, N], BF16)
        nc.gpsimd.dma_start(u_row[0:1, :], row_view(ubd))
        lnrow = sg.tile([1, N], BF16)
        nc.scalar.activation(lnrow[:], u_row[:], AF.Ln)

        # v = 1/(z + N*EPS): copy z rows to SBUF, bounce to column layout
        zrow = sg.tile([128, 1024], BF16)
        for f in range(4):
            p, cs = zsl[f]
            nc.scalar.activation(zrow[p:p + 1, cs], zps[p:p + 1, cs],
                                 AF.Copy, bias=EPS * N, scale=1.0)
            nc.sync.dma_start(
                row_view(zbd[f * 512:(f + 1) * 512]),
                zrow[p:p + 1, cs])
        zcol16 = sg.tile([128, NCH], BF16)
        nc.sync.dma_start(zcol16[:], col_view(zbd))
        v_col = sg.tile([128, NCH], F32)
        nc.vector.reciprocal(v_col[:], zcol16[:])

        # ---------------- fused CE ---------------------------------------
        m2 = sg.tile([128, 2 * NCH], F32)
        negm2 = sg.tile([128, 2 * NCH], F32)
        a2 = sg.tile([128, 2 * NCH], F32)
        for t in range(NCH):
            tsl = slice(t * 128, (t + 1) * 128)
            for h in range(2):
                sps = ps.tile([128, 1024], F32, tag="big")
                for f in range(2):
                    fs = slice(h * 1024 + f * 512, h * 1024 + (f + 1) * 512)
                    out = sps[:, f * 512:(f + 1) * 512]
                    nc.tensor.matmul(out, qT16[:, 0, tsl], kT16[:, 0, fs],
                                     start=True, stop=False)
                    nc.tensor.matmul(out, qT16[:, 1, tsl], kT16[:, 1, fs],
                                     start=False, stop=False)
                    nc.tensor.matmul(out, knTT8[:, :, tsl], qn8[:, :, fs],
                                     start=False, stop=False, perf_mode=DR,
                                     skip_group_check=True)
                    nc.tensor.matmul(out, tee_row16[:], lnrow[0:1, fs],
                                     start=False, stop=True,
                                     skip_group_check=True)
                hh = 2 * t + h
                nc.vector.tensor_reduce(m2[:, hh:hh + 1], sps[:], AX.X, ALU.max)
                nc.vector.tensor_scalar_mul(negm2[:, hh:hh + 1],
                                            m2[:, hh:hh + 1], -1.0 / T)
                es = escp.tile([128, 1024], BF16, tag="esc")
                nc.scalar.activation(es[:], sps[:], AF.Exp, scale=1.0 / T,
                                     bias=negm2[:, hh:hh + 1],
                                     accum_out=a2[:, hh:hh + 1])

        # ---------------- epilogue (column layout [128, NCH]) -------------
        m2v = m2.rearrange("p (t h) -> p t h", h=2)
        a2v = a2.rearrange("p (t h) -> p t h", h=2)
        mcol = sg.tile([128, NCH], F32)
        nc.vector.tensor_max(mcol[:], m2v[:, :, 0], m2v[:, :, 1])
        acol = sg.tile([128, NCH], F32)
        wh = sg.tile([128, NCH], F32)
        for h in range(2):
            dm = sg.tile([128, NCH], F32, tag="dm")
            nc.vector.tensor_sub(dm[:], m2v[:, :, h], mcol[:])
            eh = sg.tile([128, NCH], F32, tag="eh")
            nc.scalar.activation(eh[:], dm[:], AF.Exp, scale=1.0 / T)
            if h == 0:
                nc.vector.tensor_mul(acol[:], a2v[:, :, 0], eh[:])
            else:
                nc.vector.tensor_mul(wh[:], a2v[:, :, 1], eh[:])
        nc.vector.tensor_add(acol[:], acol[:], wh[:])

        cii = sg.tile([128, NCH], F32)
        nc.vector.tensor_mul(cii[:], sii[:], rinv_col[:, NCH:])
        nc.vector.tensor_mul(cii[:], cii[:], rinv_col[:, 0:NCH])
        ktii = sg.tile([128, NCH], F32)
        nc.scalar.activation(ktii[:], cii[:], AF.Exp, scale=-1.0)
        dcol = sg.tile([128, NCH], F32)
        nc.vector.tensor_sub(dcol[:], sii[:], mcol[:])
        epos = sg.tile([128, NCH], F32)
        nc.scalar.activation(epos[:], dcol[:], AF.Exp, scale=1.0 / T)
        diag = sg.tile([128, NCH], F32)
        nc.vector.tensor_mul(diag[:], u_col[:], ktii[:])
        nc.vector.tensor_mul(diag[:], diag[:], epos[:])
        nc.vector.tensor_sub(acol[:], acol[:], diag[:])
        nc.vector.tensor_mul(acol[:], acol[:], v_col[:])
        nc.vector.tensor_scalar_mul(acol[:], acol[:], SC)
        tot = sg.tile([128, NCH], F32)
        nc.vector.tensor_add(tot[:], acol[:], epos[:])
        lg = sg.tile([128, NCH], F32)
        nc.scalar.activation(lg[:], tot[:], AF.Ln)
        lcol = sg.tile([128, NCH], F32)
        nc.vector.tensor_scalar_mul(lcol[:], dcol[:], -1.0 / T)
        nc.vector.tensor_add(lcol[:], lcol[:], lg[:])
        nc.sync.dma_start(col_view(lossd), lcol[:])

    nc.compile()
    return nc


def _get_nc():
    global _CACHED_NC
    if _CACHED_NC is None:
        _CACHED_NC = _build()
    return _CACHED_NC


def kernel(feat_q, feat_k, current_batch):
    feat_q = np.ascontiguousarray(np.asarray(feat_q, dtype=np.float32))
    feat_k = np.ascontiguousarray(np.asarray(feat_k, dtype=np.float32))
    bb = int(current_batch)
    assert bb == 8 and feat_q.shape == (8 * N, D), (bb, feat_q.shape)

    nc = _get_nc()
    in_maps = []
    for b in range(8):
        q = feat_q[b * N:(b + 1) * N]
        k = feat_k[b * N:(b + 1) * N]
        in_maps.append({
            "qT": np.ascontiguousarray(q.T),
            "kT": np.ascontiguousarray(k.T),
        })
    res = run_bass_kernel_spmd(nc, in_maps, core_ids=list(range(8)))
    out = np.concatenate([res.results[b]["loss"].reshape(-1) for b in range(8)])
    return out.astype(np.float32)


# revision 10
# speedup vs baseline: 1.5256x; 1.1628x over previous
"""MoNCE loss (OT-regularized InfoNCE) Trainium2 kernel, v2.

Data-parallel over the 8 independent OT problems, 1 per NeuronCore.
Per core (N=2048 patches, D=256), with T = NCE temperature:

  Row stats in COLUMN layout [128,16] (cheap DVE reciprocal), bounced
  once through DRAM to row layout for a PE outer-product broadcast.
  Merged K/Sinkhorn loop (1 iteration suffices):
    per row-chunk t: cps = qn_t . (-T*kn)  [bf16 matmul]
                     K_t = exp(cps/T) [ACT, accum_out rowsum r_t]
                     u_t = 1/(r_t/N + 1e-8)  [DVE only]; u_t bounced
                     incrementally to a DRAM row
                     z += u_t^T K_t   [PE matvec -> one PSUM bank,
                                       f-slices at partitions 0/32/64/96]
    v = 1/(z + N*1e-8)
  Fused CE, one PSUM accumulation per 512-block:
    S''_ij = q_i.k_j - T*(kn_i.qn_j) + T*ln(u_j)
      q.k     : fp16 x fp16, 2 passes      (softmax max-cancellation
                makes per-term exp errors cancel; sim rel ~1e-4)
      kn.qn   : fp8e5m2 DoubleRow, 1 pass
      T*ln u  : bf16 rank-1 row
    M_i = rowmax (DVE), A_i = sum_j exp((S''-M_i)/T) [ACT accum]
    tot  = (2047/2048) v_i (A_i - u_i Ktii_i Epos_i) + Epos_i
    loss = (M_i - S_ii)/T + ln(tot)
All matmul weights are bf16/fp16/fp8 -> FWL stays enabled and the PE
HAM clock gate stays at 8/8 (the v1 kernel ran the whole CE phase at
half clock due to fp32r LDWEIGHTS).
"""

import os
from contextlib import ExitStack

import numpy as np

import concourse.bass as bass
import concourse.tile as tile
from concourse import bacc, mybir
from concourse.bass_utils import run_bass_kernel_spmd

F32 = mybir.dt.float32
F32R = mybir.dt.float32r
BF16 = mybir.dt.bfloat16
FP16 = mybir.dt.float16
FP8 = mybir.dt.float8e5
AF = mybir.ActivationFunctionType
ALU = mybir.AluOpType
AX = mybir.AxisListType
DR = mybir.MatmulPerfMode.DoubleRow

N = 2048
D = 256
NCH = N // 128    # 16 row chunks
DCH = D // 128    # 2 contraction chunks
T = 0.07
EPS = 1e-8
SC = (N - 1) / N

_CACHED_NC = None


def _build():
    nc = bacc.Bacc("TRN2", target_bir_lowering=False, debug=False, num_devices=8)

    qTd = nc.dram_tensor("qT", [D, N], F32, kind="ExternalInput").ap()
    kTd = nc.dram_tensor("kT", [D, N], F32, kind="ExternalInput").ap()
    lossd = nc.dram_tensor("loss", [N], F32, kind="ExternalOutput").ap()
    rvqd = nc.dram_tensor("rvqb", [N], BF16).ap()
    rvkd = nc.dram_tensor("rvkb", [N], BF16).ap()
    ubd = nc.dram_tensor("ub", [N], BF16).ap()
    siid = nc.dram_tensor("siib", [N], F32).ap()
    zbd = nc.dram_tensor("zb", [N], BF16).ap()

    col_view = lambda d: d.rearrange("(t p) -> p t", p=128)
    row_view = lambda d: d.rearrange("(a n) -> a n", a=1)

    with tile.TileContext(nc) as tc, ExitStack() as ctx:
        sg = ctx.enter_context(tc.tile_pool(name="sg", bufs=1))
        io = ctx.enter_context(tc.tile_pool(name="io", bufs=2))
        sqp = ctx.enter_context(tc.tile_pool(name="sqp", bufs=4))
        kcp = ctx.enter_context(tc.tile_pool(name="kcp", bufs=6))
        escp = ctx.enter_context(tc.tile_pool(name="escp", bufs=3))
        ps = ctx.enter_context(tc.tile_pool(name="ps", bufs=3, space="PSUM"))
        pl = ctx.enter_context(tc.tile_pool(name="pl", bufs=1, space="PSUM"))

        # ---------------- constants ----------------
        ones_f = sg.tile([1, 128], F32)
        nc.vector.memset(ones_f[:], 1.0)
        ones_row = sg.tile([1, 128], BF16)
        nc.vector.tensor_copy(ones_row[:], ones_f[:])
        negT_row = sg.tile([1, 128], BF16)
        nc.vector.memset(negT_row[:], -T)
        tee_row16 = sg.tile([1, 128], BF16)
        nc.vector.memset(tee_row16[:], T)
        onec_f = sg.tile([128, 1], F32)
        nc.vector.memset(onec_f[:], 1.0)
        onec_16 = sg.tile([128, 1], BF16)
        nc.vector.tensor_copy(onec_16[:], onec_f[:])
        onec_r = sg.tile([128, 1], F32R)
        nc.vector.tensor_copy(onec_r[:], onec_f[:])

        # ---------------- input DMA ----------------
        kTr = sg.tile([128, DCH, N], F32)    # raw kT
        qtch = sg.tile([128, DCH, N], F32)   # raw qT
        nc.sync.dma_start(kTr[:, 0, :], kTd[0:128, :])
        nc.scalar.dma_start(kTr[:, 1, :], kTd[128:256, :])
        nc.gpsimd.dma_start(qtch[:, 0, :], qTd[0:128, :])
        nc.sync.dma_start(qtch[:, 1, :], qTd[128:256, :])

        # squares (bf16) for column sumsq, all on ACT (idle during DMA)
        sqk = [sqp.tile([128, N], BF16, tag="sq", name=f"sqk{c}")
               for c in range(DCH)]
        sqq = [sqp.tile([128, N], BF16, tag="sq", name=f"sqq{c}")
               for c in range(DCH)]
        nc.scalar.activation(sqk[0][:], kTr[:, 0, :], AF.Square)
        nc.scalar.activation(sqk[1][:], kTr[:, 1, :], AF.Square)
        nc.scalar.activation(sqq[0][:], qtch[:, 0, :], AF.Square)
        nc.scalar.activation(sqq[1][:], qtch[:, 1, :], AF.Square)

        # fp16 casts on DVE, early (cheap 16-bit-out path); raw fp32 DVE
        # tensor ops measured ~2.6x slower, so every later big DVE op
        # takes 16-bit inputs
        qT16 = sg.tile([128, DCH, N], FP16)
        kT16 = sg.tile([128, DCH, N], FP16)
        for c in range(DCH):
            for h in range(2):
                hs = slice(h * 1024, (h + 1) * 1024)
                nc.vector.tensor_copy(kT16[:, c, hs], kTr[:, c, hs])

        # ---------------- column-layout sumsq via PE (sq^T . ones) -------
        scol = ps.tile([128, 1024], F32, tag="big", name="scol")
        # [:,0:16]=k, [16:32]=q
        for b in range(NCH):
            bs = slice(b * 128, (b + 1) * 128)
            for c in range(DCH):
                nc.tensor.matmul(scol[:, b:b + 1], sqk[c][:, bs], onec_16[:],
                                 start=(c == 0), stop=(c == DCH - 1))
        for b in range(NCH):
            bs = slice(b * 128, (b + 1) * 128)
            for c in range(DCH):
                nc.tensor.matmul(scol[:, NCH + b:NCH + b + 1], sqq[c][:, bs],
                                 onec_16[:], start=(c == 0), stop=(c == DCH - 1))

        # rinv = 1/sqrt(sumsq) in column layout (k half first), bounce to rows
        rsq_col = sg.tile([128, 2 * NCH], F32)
        rinv_col = sg.tile([128, 2 * NCH], F32)
        rinv_c16 = sg.tile([128, 2 * NCH], BF16)
        nc.vector.reciprocal(rsq_col[:, 0:NCH], scol[:, 0:NCH])
        nc.scalar.activation(rinv_col[:, 0:NCH], rsq_col[:, 0:NCH], AF.Sqrt)
        nc.vector.tensor_copy(rinv_c16[:, 0:NCH], rinv_col[:, 0:NCH])
        nc.sync.dma_start(col_view(rvkd), rinv_c16[:, 0:NCH])
        rik_row = sg.tile([1, N], BF16)
        nc.sync.dma_start(rik_row[0:1, :], row_view(rvkd))
        for c in range(DCH):
            for h in range(2):
                hs = slice(h * 1024, (h + 1) * 1024)
                nc.vector.tensor_copy(qT16[:, c, hs], qtch[:, c, hs])
        nc.vector.reciprocal(rsq_col[:, NCH:], scol[:, NCH:2 * NCH])
        nc.scalar.activation(rinv_col[:, NCH:], rsq_col[:, NCH:], AF.Sqrt)
        nc.vector.tensor_copy(rinv_c16[:, NCH:], rinv_col[:, NCH:])
        nc.sync.dma_start(col_view(rvqd), rinv_c16[:, NCH:])
        riq_row = sg.tile([1, N], BF16)
        nc.sync.dma_start(riq_row[0:1, :], row_view(rvqd))

        # broadcast rows across partitions via PE outer product
        def pe_broadcast(dst_bf16, stat_row, src_row):
            for h in range(2):
                bc = ps.tile([128, 1024], F32, tag="big")
                for f in range(2):
                    sl = slice(h * 1024 + f * 512, h * 1024 + (f + 1) * 512)
                    nc.tensor.matmul(bc[:, f * 512:(f + 1) * 512], stat_row[:],
                                     src_row[:, sl], start=True, stop=True)
                nc.scalar.copy(dst_bf16[:, h * 1024:(h + 1) * 1024], bc[:])

        rikT_bc = sg.tile([128, N], BF16, tag="bc")   # -T / |k_j|
        riq_bc = sg.tile([128, N], BF16, tag="bc")    # 1 / |q_j|
        pe_broadcast(rikT_bc, negT_row, rik_row)
        pe_broadcast(riq_bc, ones_row, riq_row)

        # ---------------- normalized features (fp8 e5m2, direct) ----------
        # knTT8 = kT16 * (-T*rinvk) ; qn8 = qT16 * rinvq -- 16-bit inputs,
        # fp8 out, used by both the K pass (plain) and CE (DoubleRow)
        qn8 = sg.tile([128, DCH, N], FP8)
        knTT8 = sg.tile([128, DCH, N], FP8)
        for c in range(DCH):
            for h in range(2):
                hs = slice(h * 1024, (h + 1) * 1024)
                nc.vector.tensor_mul(knTT8[:, c, hs], kT16[:, c, hs],
                                     rikT_bc[:, hs])
        for c in range(DCH):
            nc.vector.tensor_mul(qn8[:, c, 0:1024], qT16[:, c, 0:1024],
                                 riq_bc[:, 0:1024])

        # deferred DVE work popped one-per-chunk inside the K loop
        prod = [io.tile([128, N], F32R, tag="prod", name=f"prod{c}")
                for c in range(DCH)]
        deferred = []
        for c in range(DCH):
            deferred.append(lambda c=c: nc.vector.tensor_mul(
                qn8[:, c, 1024:2048], qT16[:, c, 1024:2048],
                riq_bc[:, 1024:2048]))
        for c in range(DCH):
            deferred.append(lambda c=c: nc.vector.tensor_mul(
                prod[c][:], qtch[:, c, :], kTr[:, c, :]))

        # ---------------- merged K pass + Sinkhorn ------------------------
        r2 = sg.tile([128, 2 * NCH], F32)     # per-half rowsums
        r_col = sg.tile([128, NCH], F32)
        u_col = sg.tile([128, NCH], F32)
        u_col16 = sg.tile([128, NCH], BF16)
        sii_r = sg.tile([1, N], F32, tag="rowtmp")
        sii = sg.tile([128, NCH], F32)
        # z f-slices: f=0,1,2 at partitions 0/32/64 (cols 0:512); f=3 at
        # partition 0, cols 512:1024 (quadrant 3 / partition 96 is unusable)
        zps = pl.tile([128, 1024], F32, tag="zps")
        zsl = [(0, slice(0, 512)), (32, slice(0, 512)),
               (64, slice(0, 512)), (0, slice(512, 1024))]

        def emit_mv(t, khs):
            for f in range(4):
                p, cs = zsl[f]
                nc.tensor.matmul(zps[p:p + 1, cs],
                                 u_col16[:, t:t + 1],
                                 khs[f // 2][:, (f % 2) * 512:(f % 2 + 1) * 512],
                                 start=(t == 0), stop=(t == NCH - 1))

        def emit_sii():
            # sii row reduce (f32r ones . prod), bounce row->col (epilogue)
            for ff in range(4):
                fs = slice(ff * 512, (ff + 1) * 512)
                pss = ps.tile([128, 1024], F32, tag="big")
                for c in range(DCH):
                    nc.tensor.matmul(pss[0:1, 0:512], onec_r[:], prod[c][:, fs],
                                     start=(c == 0), stop=(c == DCH - 1))
                nc.scalar.copy(sii_r[:, fs], pss[0:1, 0:512])
            nc.sync.dma_start(row_view(siid), sii_r[0:1, :])
            nc.sync.dma_start(sii[:], col_view(siid))

        pend = None
        for t in range(NCH):
            tsl = slice(t * 128, (t + 1) * 128)
            khs = []
            for h in range(2):
                cps = ps.tile([128, 1024], F32, tag="big")
                for f in range(2):
                    fs = slice(h * 1024 + f * 512, h * 1024 + (f + 1) * 512)
                    for c in range(DCH):
                        nc.tensor.matmul(cps[:, f * 512:(f + 1) * 512],
                                         qn8[:, c, tsl], knTT8[:, c, fs],
                                         start=(c == 0), stop=(c == DCH - 1))
                kt16 = kcp.tile([128, 1024], BF16, tag="kch")
                nc.scalar.activation(kt16[:], cps[:], AF.Exp, scale=1.0 / T,
                                     accum_out=r2[:, 2 * t + h:2 * t + h + 1])
                khs.append(kt16)
            # u for chunk t (tiny DVE column ops, no ACT)
            nc.vector.tensor_add(r_col[:, t:t + 1], r2[:, 2 * t:2 * t + 1],
                                 r2[:, 2 * t + 1:2 * t + 2])
            nc.vector.tensor_scalar(u_col[:, t:t + 1], r_col[:, t:t + 1],
                                    1.0 / N, EPS, ALU.mult, ALU.add)
            nc.vector.reciprocal(u_col[:, t:t + 1], u_col[:, t:t + 1])
            nc.vector.tensor_copy(u_col16[:, t:t + 1], u_col[:, t:t + 1])
            # bounce u chunk to DRAM row segment (gpsimd DMA ring)
            nc.gpsimd.dma_start(
                ubd[t * 128:(t + 1) * 128].rearrange("(a p) -> p a", a=1),
                u_col16[:, t:t + 1])
            # one deferred DVE op per chunk (uses DVE slack)
            if deferred:
                deferred.pop(0)()
            # matvec for the PREVIOUS chunk (u latency hidden)
            if pend is not None:
                emit_mv(*pend)
            pend = (t, khs)
            if t == 4:
                emit_sii()
        emit_mv(*pend)
        while deferred:
            deferred.pop(0)()

        # u row readback + ln(u) row (single table switch, overlaps CE start)
        u_row = sg.tile([1, N], BF16)
        nc.gpsimd.dma_start(u_row[0:1, :], row_view(ubd))
        lnrow = sg.tile([1, N], BF16)
        nc.scalar.activation(lnrow[:], u_row[:], AF.Ln)

        # ---------------- fused CE ---------------------------------------
        m2 = sg.tile([128, 2 * NCH], F32)
        negm2 = sg.tile([128, 2 * NCH], F32)
        a2 = sg.tile([128, 2 * NCH], F32)
        for t in range(NCH):
            tsl = slice(t * 128, (t + 1) * 128)
            for h in range(2):
                sps = ps.tile([128, 1024], F32, tag="big")
                # lnu matmuls last: the first chunk's q.k/DR can start
                # before the ln(u) row is ready
                for f in range(2):
                    fs = slice(h * 1024 + f * 512, h * 1024 + (f + 1) * 512)
                    out = sps[:, f * 512:(f + 1) * 512]
                    nc.tensor.matmul(out, qT16[:, 0, tsl], kT16[:, 0, fs],
                                     start=True, stop=False)
                    nc.tensor.matmul(out, qT16[:, 1, tsl], kT16[:, 1, fs],
                                     start=False, stop=False)
                    nc.tensor.matmul(out, knTT8[:, :, tsl], qn8[:, :, fs],
                                     start=False, stop=False, perf_mode=DR,
                                     skip_group_check=True)
                for f in range(2):
                    fs = slice(h * 1024 + f * 512, h * 1024 + (f + 1) * 512)
                    out = sps[:, f * 512:(f + 1) * 512]
                    nc.tensor.matmul(out, tee_row16[:], lnrow[0:1, fs],
                                     start=False, stop=True,
                                     skip_group_check=True)
                hh = 2 * t + h
                nc.vector.tensor_reduce(m2[:, hh:hh + 1], sps[:], AX.X, ALU.max)
                nc.vector.tensor_scalar_mul(negm2[:, hh:hh + 1],
                                            m2[:, hh:hh + 1], -1.0 / T)
                es = escp.tile([128, 1024], BF16, tag="esc")
                nc.scalar.activation(es[:], sps[:], AF.Exp, scale=1.0 / T,
                                     bias=negm2[:, hh:hh + 1],
                                     accum_out=a2[:, hh:hh + 1])

        # v = 1/(z + N*EPS): off the ACT critical path (epilogue-only use)
        zrow = sg.tile([128, 1024], BF16)
        for f in range(4):
            p, cs = zsl[f]
            nc.scalar.activation(zrow[p:p + 1, cs], zps[p:p + 1, cs],
                                 AF.Copy, bias=EPS * N, scale=1.0)
            nc.sync.dma_start(
                row_view(zbd[f * 512:(f + 1) * 512]),
                zrow[p:p + 1, cs])
        zcol16 = sg.tile([128, NCH], BF16)
        nc.sync.dma_start(zcol16[:], col_view(zbd))
        v_col = sg.tile([128, NCH], F32)
        nc.vector.reciprocal(v_col[:], zcol16[:])

        # ---------------- epilogue (column layout [128, NCH]) -------------
        m2v = m2.rearrange("p (t h) -> p t h", h=2)
        a2v = a2.rearrange("p (t h) -> p t h", h=2)
        mcol = sg.tile([128, NCH], F32)
        nc.vector.tensor_max(mcol[:], m2v[:, :, 0], m2v[:, :, 1])
        acol = sg.tile([128, NCH], F32)
        wh = sg.tile([128, NCH], F32)
        for h in range(2):
            dm = sg.tile([128, NCH], F32, tag="dm")
            nc.vector.tensor_sub(dm[:], m2v[:, :, h], mcol[:])
            eh = sg.tile([128, NCH], F32, tag="eh")
            nc.scalar.activation(eh[:], dm[:], AF.Exp, scale=1.0 / T)
            if h == 0:
                nc.vector.tensor_mul(acol[:], a2v[:, :, 0], eh[:])
            else:
                nc.vector.tensor_mul(wh[:], a2v[:, :, 1], eh[:])
        nc.vector.tensor_add(acol[:], acol[:], wh[:])

        cii = sg.tile([128, NCH], F32)
        nc.vector.tensor_mul(cii[:], sii[:], rinv_col[:, NCH:])
        nc.vector.tensor_mul(cii[:], cii[:], rinv_col[:, 0:NCH])
        ktii = sg.tile([128, NCH], F32)
        nc.scalar.activation(ktii[:], cii[:], AF.Exp, scale=-1.0)
        dcol = sg.tile([128, NCH], F32)
        nc.vector.tensor_sub(dcol[:], sii[:], mcol[:])
        epos = sg.tile([128, NCH], F32)
        nc.scalar.activation(epos[:], dcol[:], AF.Exp, scale=1.0 / T)
        diag = sg.tile([128, NCH], F32)
        nc.vector.tensor_mul(diag[:], u_col[:], ktii[:])
        nc.vector.tensor_mul(diag[:], diag[:], epos[:])
        nc.vector.tensor_sub(acol[:], acol[:], diag[:])
        nc.vector.tensor_mul(acol[:], acol[:], v_col[:])
        nc.vector.tensor_scalar_mul(acol[:], acol[:], SC)
        tot = sg.tile([128, NCH], F32)
        nc.vector.tensor_add(tot[:], acol[:], epos[:])
        lg = sg.tile([128, NCH], F32)
        nc.scalar.activation(lg[:], tot[:], AF.Ln)
        lcol = sg.tile([128, NCH], F32)
        nc.vector.tensor_scalar_mul(lcol[:], dcol[:], -1.0 / T)
        nc.vector.tensor_add(lcol[:], lcol[:], lg[:])
        nc.sync.dma_start(col_view(lossd), lcol[:])

    nc.compile()
    return nc


def _get_nc():
    global _CACHED_NC
    if _CACHED_NC is None:
        _CACHED_NC = _build()
    return _CACHED_NC


def kernel(feat_q, feat_k, current_batch):
    feat_q = np.ascontiguousarray(np.asarray(feat_q, dtype=np.float32))
    feat_k = np.ascontiguousarray(np.asarray(feat_k, dtype=np.float32))
    bb = int(current_batch)
    assert bb == 8 and feat_q.shape == (8 * N, D), (bb, feat_q.shape)

    nc = _get_nc()
    in_maps = []
    for b in range(8):
        q = feat_q[b * N:(b + 1) * N]
        k = feat_k[b * N:(b + 1) * N]
        in_maps.append({
            "qT": np.ascontiguousarray(q.T),
            "kT": np.ascontiguousarray(k.T),
        })
    res = run_bass_kernel_spmd(nc, in_maps, core_ids=list(range(8)))
    out = np.concatenate([res.results[b]["loss"].reshape(-1) for b in range(8)])
    return out.astype(np.float32)


# revision 11
# speedup vs baseline: 1.9479x; 1.2769x over previous
"""MoNCE loss (OT-regularized InfoNCE) Trainium2 kernel, v4.

Data-parallel over the 8 independent OT problems, 1 per NeuronCore.
Per core (N=2048 patches, D=256), with T = NCE temperature:

  Row stats in COLUMN layout [128,16] (cheap DVE reciprocal). Layout
  conversions NEVER use scattered element DMA (a [128,16] col-view DMA
  was measured at 9-18us transfer): instead DVE 32x32 block-transpose
  + 4 contiguous-line DMAs (write side), or a [16,128] contiguous read
  + PE transpose with a 16x16 identity (read side).
  Merged K/Sinkhorn loop (1 iteration suffices):
    per row-chunk t: cps = qn_t . (-T*kn) via fp8e5m2 matmuls
                     K_t = exp(cps/T) [ACT, accum_out rowsum r_t]
                     u_t = 1/(r_t/N + 1e-8)  [DVE only]
                     z += u_t^T K_t   [PE matvec -> one PSUM tile,
                                       f-slices at partitions 0/32/64]
    u bounced to a DRAM row once at K end (block-transpose path)
    v = 1/(z + N*1e-8)
  Fused CE, one PSUM accumulation per 512-block:
    S''_ij = q_i.k_j - T*(kn_i.qn_j) + T*ln(u_j)
      q.k     : fp16 x fp16, 2 passes      (softmax max-cancellation
                makes per-term exp errors cancel; sim rel ~1e-4)
      kn.qn   : fp8e5m2 DoubleRow, 1 pass
      T*ln u  : bf16 rank-1 row (emitted after q.k/DR so chunk 0
                overlaps the ln(u) tail)
    M_i = rowmax (DVE), A_i = sum_j exp((S''-M_i)/T) [ACT accum]
    tot  = (2047/2048) v_i (A_i - u_i Ktii_i Epos_i) + Epos_i
    loss = (M_i - S_ii)/T + ln(tot)
All matmul weights are bf16/fp16/fp8 -> FWL stays enabled and the PE
HAM clock gate stays at 8/8.
"""

import os
from contextlib import ExitStack

import numpy as np

import concourse.bass as bass
import concourse.tile as tile
from concourse import bacc, mybir
from concourse.bass_utils import run_bass_kernel_spmd
from concourse.masks import make_identity

F32 = mybir.dt.float32
F32R = mybir.dt.float32r
BF16 = mybir.dt.bfloat16
FP16 = mybir.dt.float16
FP8 = mybir.dt.float8e5
AF = mybir.ActivationFunctionType
ALU = mybir.AluOpType
AX = mybir.AxisListType
DR = mybir.MatmulPerfMode.DoubleRow

N = 2048
D = 256
NCH = N // 128    # 16 row chunks
DCH = D // 128    # 2 contraction chunks
T = 0.07
EPS = 1e-8
SC = (N - 1) / N

_CACHED_NC = None


def _build():
    nc = bacc.Bacc("TRN2", target_bir_lowering=False, debug=False, num_devices=8)

    qTd = nc.dram_tensor("qT", [D, N], F32, kind="ExternalInput").ap()
    kTd = nc.dram_tensor("kT", [D, N], F32, kind="ExternalInput").ap()
    lossd = nc.dram_tensor("loss", [N], F32, kind="ExternalOutput").ap()
    rvqd = nc.dram_tensor("rvqb", [N], BF16).ap()
    rvkd = nc.dram_tensor("rvkb", [N], BF16).ap()
    ubd = nc.dram_tensor("ub", [N], BF16).ap()
    siid = nc.dram_tensor("siib", [N], F32).ap()
    zbd = nc.dram_tensor("zb", [N], BF16).ap()

    row_view = lambda d: d.rearrange("(a n) -> a n", a=1)
    # [16,128] per-partition-contiguous view: partition u -> offsets u*128..
    t16_view = lambda d: d.rearrange("(u q) -> u q", q=128)

    with tile.TileContext(nc) as tc, ExitStack() as ctx:
        sg = ctx.enter_context(tc.tile_pool(name="sg", bufs=1))
        io = ctx.enter_context(tc.tile_pool(name="io", bufs=2))
        sqp = ctx.enter_context(tc.tile_pool(name="sqp", bufs=4))
        kcp = ctx.enter_context(tc.tile_pool(name="kcp", bufs=6))
        escp = ctx.enter_context(tc.tile_pool(name="escp", bufs=3))
        ps = ctx.enter_context(tc.tile_pool(name="ps", bufs=3, space="PSUM"))
        pl = ctx.enter_context(tc.tile_pool(name="pl", bufs=1, space="PSUM"))

        # ---------------- constants ----------------
        ones_f = sg.tile([1, 128], F32)
        nc.vector.memset(ones_f[:], 1.0)
        ones_row = sg.tile([1, 128], BF16)
        nc.vector.tensor_copy(ones_row[:], ones_f[:])
        negT_row = sg.tile([1, 128], BF16)
        nc.vector.memset(negT_row[:], -T)
        tee_row16 = sg.tile([1, 128], BF16)
        nc.vector.memset(tee_row16[:], T)
        onec_f = sg.tile([128, 1], F32)
        nc.vector.memset(onec_f[:], 1.0)
        onec_16 = sg.tile([128, 1], BF16)
        nc.vector.tensor_copy(onec_16[:], onec_f[:])
        onec_r = sg.tile([128, 1], F32R)
        nc.vector.tensor_copy(onec_r[:], onec_f[:])
        idn16f = sg.tile([16, 16], F32)
        make_identity(nc, idn16f[:])
        idn16b = sg.tile([16, 16], BF16)
        make_identity(nc, idn16b[:])

        # ---------------- input DMA (k first: it gates the K pass) -------
        kTr = sg.tile([128, DCH, N], F32)
        qtch = sg.tile([128, DCH, N], F32)
        nc.sync.dma_start(kTr[:, 0, :], kTd[0:128, :])
        nc.scalar.dma_start(kTr[:, 1, :], kTd[128:256, :])
        nc.gpsimd.dma_start(qtch[:, 0, :], qTd[0:128, :])
        nc.sync.dma_start(qtch[:, 1, :], qTd[128:256, :])

        # squares (bf16): k + sqq0 on ACT, sqq1 on GpSimd (parallel)
        sqk = [sqp.tile([128, N], BF16, tag="sq", name=f"sqk{c}")
               for c in range(DCH)]
        sqq = [sqp.tile([128, N], BF16, tag="sq", name=f"sqq{c}")
               for c in range(DCH)]
        nc.scalar.activation(sqk[0][:], kTr[:, 0, :], AF.Square)
        nc.scalar.activation(sqk[1][:], kTr[:, 1, :], AF.Square)
        nc.scalar.activation(sqq[0][:], qtch[:, 0, :], AF.Square)
        nc.gpsimd.tensor_mul(sqq[1][:], qtch[:, 1, :], qtch[:, 1, :])

        # fp16 casts (cheap single-src DVE path), k side first
        qT16 = sg.tile([128, DCH, N], FP16)
        kT16 = sg.tile([128, DCH, N], FP16)
        for c in range(DCH):
            for h in range(2):
                hs = slice(h * 1024, (h + 1) * 1024)
                nc.vector.tensor_copy(kT16[:, c, hs], kTr[:, c, hs])

        # ---------------- column-layout sumsq via PE (sq^T . ones) -------
        scol = ps.tile([128, 1024], F32, tag="big", name="scol")
        # [:,0:16]=k, [16:32]=q
        for b in range(NCH):
            bs = slice(b * 128, (b + 1) * 128)
            for c in range(DCH):
                nc.tensor.matmul(scol[:, b:b + 1], sqk[c][:, bs], onec_16[:],
                                 start=(c == 0), stop=(c == DCH - 1))

        # k-side rinv: recip+sqrt in col layout, block-transpose, 4
        # contiguous DMAs, one contiguous row read
        rsq_col = sg.tile([128, 2 * NCH], F32)
        rinv_col = sg.tile([128, 2 * NCH], F32)
        rvk16 = sg.tile([128, 32], BF16)   # cols 0:16 valid
        rvq16 = sg.tile([128, 32], BF16)
        btk = sg.tile([128, 32], BF16)
        btq = sg.tile([128, 32], BF16)
        nc.vector.reciprocal(rsq_col[:, 0:NCH], scol[:, 0:NCH])
        nc.scalar.activation(rinv_col[:, 0:NCH], rsq_col[:, 0:NCH], AF.Sqrt)
        nc.vector.tensor_copy(rvk16[:, 0:NCH], rinv_col[:, 0:NCH])
        nc.vector.transpose(btk[:], rvk16[:])
        for g in range(4):
            nc.sync.dma_start(t16_view(rvkd)[:, 32 * g:32 * g + 32],
                              btk[32 * g:32 * g + 16, :])
        rik_row = sg.tile([1, N], BF16)
        nc.sync.dma_start(rik_row[0:1, :], row_view(rvkd))

        # broadcast across partitions via PE outer product
        def pe_broadcast(dst_bf16, stat_row, src_row):
            for h in range(2):
                bc = ps.tile([128, 1024], F32, tag="big")
                for f in range(2):
                    sl = slice(h * 1024 + f * 512, h * 1024 + (f + 1) * 512)
                    nc.tensor.matmul(bc[:, f * 512:(f + 1) * 512], stat_row[:],
                                     src_row[:, sl], start=True, stop=True)
                nc.vector.tensor_copy(dst_bf16[:, h * 1024:(h + 1) * 1024],
                                      bc[:])

        rikT_bc = sg.tile([128, N], BF16, tag="bc")   # -T / |k_j|
        riq_bc = sg.tile([128, N], BF16, tag="bc")    # 1 / |q_j|
        pe_broadcast(rikT_bc, negT_row, rik_row)

        # knTT8 = kT16 * (-T*rinvk) -> fp8 e5m2 (c0 DVE, c1 GpSimd)
        qn8 = sg.tile([128, DCH, N], FP8)
        knTT8 = sg.tile([128, DCH, N], FP8)
        for h in range(2):
            hs = slice(h * 1024, (h + 1) * 1024)
            nc.vector.tensor_mul(knTT8[:, 0, hs], kT16[:, 0, hs],
                                 rikT_bc[:, hs])
            nc.gpsimd.tensor_mul(knTT8[:, 1, hs], kT16[:, 1, hs],
                                 rikT_bc[:, hs])

        # q-side rinv chain (colsum-q waits sqq; emitted after k on PE)
        for b in range(NCH):
            bs = slice(b * 128, (b + 1) * 128)
            for c in range(DCH):
                nc.tensor.matmul(scol[:, NCH + b:NCH + b + 1], sqq[c][:, bs],
                                 onec_16[:], start=(c == 0), stop=(c == DCH - 1))
        nc.vector.reciprocal(rsq_col[:, NCH:], scol[:, NCH:2 * NCH])
        nc.scalar.activation(rinv_col[:, NCH:], rsq_col[:, NCH:], AF.Sqrt)
        nc.vector.tensor_copy(rvq16[:, 0:NCH], rinv_col[:, NCH:])
        nc.vector.transpose(btq[:], rvq16[:])
        for g in range(4):
            nc.gpsimd.dma_start(t16_view(rvqd)[:, 32 * g:32 * g + 32],
                                btq[32 * g:32 * g + 16, :])
        riq_row = sg.tile([1, N], BF16)
        nc.gpsimd.dma_start(riq_row[0:1, :], row_view(rvqd))
        pe_broadcast(riq_bc, ones_row, riq_row)

        # qT16 casts + qn8 h0 muls (h1 deferred into the K loop)
        for c in range(DCH):
            for h in range(2):
                hs = slice(h * 1024, (h + 1) * 1024)
                nc.vector.tensor_copy(qT16[:, c, hs], qtch[:, c, hs])
        for c in range(DCH):
            nc.vector.tensor_mul(qn8[:, c, 0:1024], qT16[:, c, 0:1024],
                                 riq_bc[:, 0:1024])

        prod = [io.tile([128, N], F32R, tag="prod", name=f"prod{c}")
                for c in range(DCH)]
        deferred = [None, None]  # two idle chunks before DVE extras
        for c in range(DCH):
            deferred.append(lambda c=c: nc.vector.tensor_mul(
                qn8[:, c, 1024:2048], qT16[:, c, 1024:2048],
                riq_bc[:, 1024:2048]))
        for c in range(DCH):
            deferred.append(lambda c=c: nc.vector.tensor_mul(
                prod[c][:], qtch[:, c, :], kTr[:, c, :]))

        # ---------------- merged K pass + Sinkhorn ------------------------
        r2 = sg.tile([128, 2 * NCH], F32)
        r_col = sg.tile([128, NCH], F32)
        u_col = sg.tile([128, NCH], F32)
        u_col16 = sg.tile([128, 32], BF16)   # cols 0:16 valid (pad for bt)
        sii_r = sg.tile([1, N], F32, tag="rowtmp")
        # z f-slices: f=0,1,2 at partitions 0/32/64 (cols 0:512); f=3 at
        # partition 0, cols 512:1024 (partition 96 is unusable)
        zps = pl.tile([128, 1024], F32, tag="zps")
        zsl = [(0, slice(0, 512)), (32, slice(0, 512)),
               (64, slice(0, 512)), (0, slice(512, 1024))]

        def emit_mv(t, khs):
            for f in range(4):
                p, cs = zsl[f]
                nc.tensor.matmul(zps[p:p + 1, cs],
                                 u_col16[:, t:t + 1],
                                 khs[f // 2][:, (f % 2) * 512:(f % 2 + 1) * 512],
                                 start=(t == 0), stop=(t == NCH - 1))

        def emit_sii():
            # sii row reduce (f32r ones . prod); row write only (contiguous)
            for ff in range(4):
                fs = slice(ff * 512, (ff + 1) * 512)
                pss = ps.tile([128, 1024], F32, tag="big")
                for c in range(DCH):
                    nc.tensor.matmul(pss[0:1, 0:512], onec_r[:], prod[c][:, fs],
                                     start=(c == 0), stop=(c == DCH - 1))
                nc.scalar.copy(sii_r[:, fs], pss[0:1, 0:512])
            nc.sync.dma_start(row_view(siid), sii_r[0:1, :])

        pend = None
        for t in range(NCH):
            tsl = slice(t * 128, (t + 1) * 128)
            khs = []
            for h in range(2):
                cps = ps.tile([128, 1024], F32, tag="big")
                for f in range(2):
                    fs = slice(h * 1024 + f * 512, h * 1024 + (f + 1) * 512)
                    for c in range(DCH):
                        nc.tensor.matmul(cps[:, f * 512:(f + 1) * 512],
                                         qn8[:, c, tsl], knTT8[:, c, fs],
                                         start=(c == 0), stop=(c == DCH - 1))
                kt16 = kcp.tile([128, 1024], BF16, tag="kch")
                nc.scalar.activation(kt16[:], cps[:], AF.Exp, scale=1.0 / T,
                                     accum_out=r2[:, 2 * t + h:2 * t + h + 1])
                khs.append(kt16)
            # u for chunk t (tiny DVE column ops only)
            nc.vector.tensor_add(r_col[:, t:t + 1], r2[:, 2 * t:2 * t + 1],
                                 r2[:, 2 * t + 1:2 * t + 2])
            nc.vector.tensor_scalar(u_col[:, t:t + 1], r_col[:, t:t + 1],
                                    1.0 / N, EPS, ALU.mult, ALU.add)
            nc.vector.reciprocal(u_col[:, t:t + 1], u_col[:, t:t + 1])
            nc.vector.tensor_copy(u_col16[:, t:t + 1], u_col[:, t:t + 1])
            if deferred:
                d = deferred.pop(0)
                if d is not None:
                    d()
            # matvec for the PREVIOUS chunk (u latency hidden)
            if pend is not None:
                emit_mv(*pend)
            pend = (t, khs)
            if t == 6:
                emit_sii()
        emit_mv(*pend)

        # u -> DRAM row via block-transpose (contiguous DMA lines) + ln(u)
        ubt = sg.tile([128, 32], BF16)
        nc.vector.transpose(ubt[:], u_col16[:])
        for g in range(4):
            nc.gpsimd.dma_start(t16_view(ubd)[:, 32 * g:32 * g + 32],
                                ubt[32 * g:32 * g + 16, :])
        u_row = sg.tile([1, N], BF16)
        nc.gpsimd.dma_start(u_row[0:1, :], row_view(ubd))
        lnrow = sg.tile([1, N], BF16)
        nc.scalar.activation(lnrow[:], u_row[:], AF.Ln)

        # ---------------- fused CE ---------------------------------------
        m2 = sg.tile([128, 2 * NCH], F32)
        negm2 = sg.tile([128, 2 * NCH], F32)
        a2 = sg.tile([128, 2 * NCH], F32)
        for t in range(NCH):
            tsl = slice(t * 128, (t + 1) * 128)
            for h in range(2):
                sps = ps.tile([128, 1024], F32, tag="big")
                for f in range(2):
                    fs = slice(h * 1024 + f * 512, h * 1024 + (f + 1) * 512)
                    out = sps[:, f * 512:(f + 1) * 512]
                    nc.tensor.matmul(out, qT16[:, 0, tsl], kT16[:, 0, fs],
                                     start=True, stop=False)
                    nc.tensor.matmul(out, qT16[:, 1, tsl], kT16[:, 1, fs],
                                     start=False, stop=False)
                    nc.tensor.matmul(out, knTT8[:, :, tsl], qn8[:, :, fs],
                                     start=False, stop=False, perf_mode=DR,
                                     skip_group_check=True)
                for f in range(2):
                    fs = slice(h * 1024 + f * 512, h * 1024 + (f + 1) * 512)
                    out = sps[:, f * 512:(f + 1) * 512]
                    nc.tensor.matmul(out, tee_row16[:], lnrow[0:1, fs],
                                     start=False, stop=True,
                                     skip_group_check=True)
                hh = 2 * t + h
                nc.vector.tensor_reduce(m2[:, hh:hh + 1], sps[:], AX.X, ALU.max)
                nc.vector.tensor_scalar_mul(negm2[:, hh:hh + 1],
                                            m2[:, hh:hh + 1], -1.0 / T)
                es = escp.tile([128, 1024], BF16, tag="esc")
                nc.scalar.activation(es[:], sps[:], AF.Exp, scale=1.0 / T,
                                     bias=negm2[:, hh:hh + 1],
                                     accum_out=a2[:, hh:hh + 1])

        # ---------------- read-backs: sii and z to column layout ----------
        siiT = sg.tile([16, 128], F32)
        nc.sync.dma_start(siiT[:], t16_view(siid))
        tps = ps.tile([128, 1024], F32, tag="big", name="tps")
        nc.tensor.transpose(tps[:, 0:NCH], siiT[:], idn16f[:])
        sii = sg.tile([128, NCH], F32)
        nc.vector.tensor_copy(sii[:], tps[:, 0:NCH])

        # v = 1/(z + N*EPS)
        zrow = sg.tile([128, 1024], BF16)
        for f in range(4):
            p, cs = zsl[f]
            nc.scalar.activation(zrow[p:p + 1, cs], zps[p:p + 1, cs],
                                 AF.Copy, bias=EPS * N, scale=1.0)
            nc.sync.dma_start(
                row_view(zbd[f * 512:(f + 1) * 512]),
                zrow[p:p + 1, cs])
        zT = sg.tile([16, 128], BF16)
        nc.sync.dma_start(zT[:], t16_view(zbd))
        tpz = ps.tile([128, 1024], BF16, tag="big", name="tpz")
        nc.tensor.transpose(tpz[:, 0:NCH], zT[:], idn16b[:])
        v_col = sg.tile([128, NCH], F32)
        nc.vector.reciprocal(v_col[:], tpz[:, 0:NCH])

        # ---------------- epilogue (column layout [128, NCH]) -------------
        m2v = m2.rearrange("p (t h) -> p t h", h=2)
        a2v = a2.rearrange("p (t h) -> p t h", h=2)
        mcol = sg.tile([128, NCH], F32)
        nc.vector.tensor_max(mcol[:], m2v[:, :, 0], m2v[:, :, 1])
        acol = sg.tile([128, NCH], F32)
        wh = sg.tile([128, NCH], F32)
        for h in range(2):
            dm = sg.tile([128, NCH], F32, tag="dm")
            nc.vector.tensor_sub(dm[:], m2v[:, :, h], mcol[:])
            eh = sg.tile([128, NCH], F32, tag="eh")
            nc.scalar.activation(eh[:], dm[:], AF.Exp, scale=1.0 / T)
            if h == 0:
                nc.vector.tensor_mul(acol[:], a2v[:, :, 0], eh[:])
            else:
                nc.vector.tensor_mul(wh[:], a2v[:, :, 1], eh[:])
        nc.vector.tensor_add(acol[:], acol[:], wh[:])

        cii = sg.tile([128, NCH], F32)
        nc.vector.tensor_mul(cii[:], sii[:], rinv_col[:, NCH:])
        nc.vector.tensor_mul(cii[:], cii[:], rinv_col[:, 0:NCH])
        ktii = sg.tile([128, NCH], F32)
        nc.scalar.activation(ktii[:], cii[:], AF.Exp, scale=-1.0)
        dcol = sg.tile([128, NCH], F32)
        nc.vector.tensor_sub(dcol[:], sii[:], mcol[:])
        epos = sg.tile([128, NCH], F32)
        nc.scalar.activation(epos[:], dcol[:], AF.Exp, scale=1.0 / T)
        diag = sg.tile([128, NCH], F32)
        nc.vector.tensor_mul(diag[:], u_col[:], ktii[:])
        nc.vector.tensor_mul(diag[:], diag[:], epos[:])
        nc.vector.tensor_sub(acol[:], acol[:], diag[:])
        nc.vector.tensor_mul(acol[:], acol[:], v_col[:])
        nc.vector.tensor_scalar_mul(acol[:], acol[:], SC)
        tot = sg.tile([128, NCH], F32)
        nc.vector.tensor_add(tot[:], acol[:], epos[:])
        lg = sg.tile([128, NCH], F32)
        nc.scalar.activation(lg[:], tot[:], AF.Ln)
        lout = sg.tile([128, 32], F32)   # cols 0:16 valid (pad for bt)
        nc.vector.tensor_scalar_mul(lout[:, 0:NCH], dcol[:], -1.0 / T)
        nc.vector.tensor_add(lout[:, 0:NCH], lout[:, 0:NCH], lg[:])

        # loss out via block-transpose + 4 contiguous DMAs (fp32 exact)
        lbt = sg.tile([128, 32], F32)
        nc.vector.transpose(lbt[:], lout[:])
        for g in range(4):
            eng = nc.sync if g % 2 == 0 else nc.gpsimd
            eng.dma_start(t16_view(lossd)[:, 32 * g:32 * g + 32],
                          lbt[32 * g:32 * g + 16, :])

    nc.compile()
    return nc


def _get_nc():
    global _CACHED_NC
    if _CACHED_NC is None:
        _CACHED_NC = _build()
    return _CACHED_NC


def kernel(feat_q, feat_k, current_batch):
    feat_q = np.ascontiguousarray(np.asarray(feat_q, dtype=np.float32))
    feat_k = np.ascontiguousarray(np.asarray(feat_k, dtype=np.float32))
    bb = int(current_batch)
    assert bb == 8 and feat_q.shape == (8 * N, D), (bb, feat_q.shape)

    nc = _get_nc()
    in_maps = []
    for b in range(8):
        q = feat_q[b * N:(b + 1) * N]
        k = feat_k[b * N:(b + 1) * N]
        in_maps.append({
            "qT": np.ascontiguousarray(q.T),
            "kT": np.ascontiguousarray(k.T),
        })
    res = run_bass_kernel_spmd(nc, in_maps, core_ids=list(range(8)))
    out = np.concatenate([res.results[b]["loss"].reshape(-1) for b in range(8)])
    return out.astype(np.float32)
